# revision 36
# baseline (speedup 1.0000x reference)
"""Multi-head attention (16 heads, B=4, L=1024, D=1024) on 8 TRN2 NeuronCores.

Sharding: core c = (batch b = c//2, head-half = c%2). Each core computes, for
its batch, the Q/K/V projections restricted to its 512 output columns
(8 heads), full attention for those heads over the batch's 1024 keys, and the
0.5*q + 0.5*ctx blend for its [1024, 512] output slice.

Device matmuls run in transposed layouts (contraction dim on partitions).
Q/K/scores use float32r operands (full PE rate at N>=512, ~1.5e-4 precision —
needed because softmax exponentiates score errors). V/expT/ctx use bf16
(attention weights and values tolerate 0.4% rounding).

Schedule: DMAs stream in consumption order (xk/xq, then the m=0 weight
columns so the first head-pair's scores start ~25us in, then remaining
weights, then V weights); Q/K projections track DMA arrivals through a
4-slot 1-bank PSUM rotation; scores for head pairs are packed into PE
row-groups 0-63/64-127 (K=64 concurrency); exp evictions (the ACT
bottleneck, ~74us) pipeline against ctx matmuls one head behind.

Per-core layout:
  QT [d' 512, q 1024], KT [d' 512, kt 1024] (proj transposed, relu+bias)
  V_aug [kt 1024, 520]; per head h: cols h*65..h*65+63 = V values,
     col h*65+64 = 2.0 (from bv_aug via the ones-row bias matmul), so the ctx
     matmul also produces a 2*sum(exp) row per head (flash-style).
  scoresT [kt, q] per head -> exp (no max-sub; scores in [0, 42]) -> expT
  ctxT_aug [65, q] per head; row 64 = 2*sumexp
  out = 0.5*q + ctx/(2*sumexp)   (residual pre-halved on host)
"""
import sys

sys.path.insert(0, "/opt/trn_rl_repo")

import numpy as np


def _build(nc_mod):
    bass, mybir, tile, bacc = nc_mod
    f32 = mybir.dt.float32
    f32r = mybir.dt.float32r
    bf16 = mybir.dt.bfloat16
    AF = mybir.ActivationFunctionType
    ALU = mybir.AluOpType

    D = 1024        # model dim / contraction dim
    DS = 512        # per-core output-column slice
    DSA = DS + 8    # with one aug column per head
    L = 1024        # sequence length (q and kt)
    KO = D // 128   # k chunks
    MQ = DS // 128  # m-chunks of d' slice (4)
    NQ = L // 512   # n-chunks of seq (2)
    NH = 8          # heads per core
    DH = 64

    nc = bacc.Bacc("TRN2", target_bir_lowering=False, debug=False)
    with tile.TileContext(nc) as tc:
        with (
            tc.tile_pool(name="dram", bufs=1, space="DRAM") as dram,
            tc.tile_pool(name="persist", bufs=1) as sp,
            tc.tile_pool(name="expp", bufs=4) as ep,
            tc.tile_pool(name="pp1", bufs=4, space="PSUM") as pp1,
            tc.tile_pool(name="pp_sc", bufs=2, space="PSUM") as pp_sc,
        ):
            # ---- I/O ----
            xqT = dram.tile([D, L], f32r, kind="ExternalInput", name="xqT")
            xkT = dram.tile([D, L], f32r, kind="ExternalInput", name="xkT")
            wq = dram.tile([D, DS], f32r, kind="ExternalInput", name="wq")
            wk = dram.tile([D, DS], f32r, kind="ExternalInput", name="wk")
            wv = dram.tile([D, DSA], f32r, kind="ExternalInput", name="wv")
            bq = dram.tile([128, MQ], f32, kind="ExternalInput", name="bq")
            bk = dram.tile([128, MQ], f32, kind="ExternalInput", name="bk")
            bv = dram.tile([1, DSA], f32r, kind="ExternalInput", name="bv")
            ones = dram.tile([1, 128], f32r, kind="ExternalInput", name="ones")
            xqh = dram.tile([DS, L], f32, kind="ExternalInput", name="xqh")
            outT = dram.tile([DS, L], f32, kind="ExternalOutput", name="outT")

            # ---- persistent SBUF ----
            qt_all = sp.tile([128, MQ, L], f32r)
            kt_all = sp.tile([128, MQ, L], f32r)
            v_all = sp.tile([128, KO, DSA], bf16)

            with tc.tile_pool(name="xw", bufs=1) as xw:
                bq_sb = xw.tile([128, MQ], f32)
                bk_sb = xw.tile([128, MQ], f32)
                bv_sb = xw.tile([1, DSA], f32r)
                ones_sb = xw.tile([1, 128], f32r)
                nc.sync.dma_start(bq_sb[:], bq[:])
                nc.sync.dma_start(bk_sb[:], bk[:])
                nc.sync.dma_start(bv_sb[:], bv[:])
                nc.sync.dma_start(ones_sb[:], ones[:])

                # preload the exp ACT table during the DMA phase
                dmy = xw.tile([1, 8], f32)
                nc.vector.memset(dmy[:], 0.0)
                dmy2 = xw.tile([1, 8], f32)
                nc.scalar.activation(dmy2[:], dmy[:], AF.Exp)

                xq_t, xk_t, wq_t, wk_t, wv_t = ([None] * KO for _ in range(5))
                # stream order: (xk, wv) first so the V projection runs during
                # the DMA phase; then (xq, m0-weight-columns) so head-pair 0
                # scores start right after; then the remaining weight columns
                for k in range(KO):
                    xk_t[k] = xw.tile([128, L], f32r, tag=f"xk{k}", name=f"xk_{k}")
                    nc.sync.dma_start(xk_t[k][:], xkT[k * 128:(k + 1) * 128, :])
                    wv_t[k] = xw.tile([128, DSA], f32r, tag=f"wv{k}", name=f"wv_{k}")
                    nc.sync.dma_start(wv_t[k][:], wv[k * 128:(k + 1) * 128, :])
                for k in range(KO):
                    xq_t[k] = xw.tile([128, L], f32r, tag=f"xq{k}", name=f"xq_{k}")
                    nc.sync.dma_start(xq_t[k][:], xqT[k * 128:(k + 1) * 128, :])
                    wk_t[k] = xw.tile([128, DS], f32r, tag=f"wk{k}", name=f"wk_{k}")
                    nc.sync.dma_start(wk_t[k][:, 0:128], wk[k * 128:(k + 1) * 128, 0:128])

                # V: out[kt 128, 520] = sum_k XkT[k,ktchunk].T @ Wv_aug[k,:]
                #    + ones.T @ bv_aug  (bias + the 2.0 aug columns)
                vtail = pp1.tile([128, KO * 8], f32, tag="p1", name="vtail")
                for t in range(KO):
                    psb = pp1.tile([128, 512], f32, tag="p1", name=f"pv{t}")
                    for k in range(KO):
                        nc.tensor.matmul(
                            psb[:], xk_t[k][:, t * 128:(t + 1) * 128],
                            wv_t[k][:, 0:512], start=(k == 0), stop=False,
                        )
                    nc.tensor.matmul(psb[:], ones_sb[:], bv_sb[:, 0:512],
                                     start=False, stop=True)
                    nc.vector.tensor_scalar(
                        v_all[:, t, 0:512], psb[:], 0.0, None, ALU.max,
                    )
                    tl = vtail[:, t * 8:(t + 1) * 8]
                    for k in range(KO):
                        nc.tensor.matmul(
                            tl, xk_t[k][:, t * 128:(t + 1) * 128],
                            wv_t[k][:, 512:DSA], start=(k == 0), stop=False,
                        )
                    nc.tensor.matmul(tl, ones_sb[:], bv_sb[:, 512:DSA],
                                     start=False, stop=True)
                nc.vector.tensor_scalar(
                    v_all[:, :, 512:DSA],
                    vtail[:].rearrange("p (t e) -> p t e", e=8),
                    0.0, None, ALU.max,
                )

                # wq reuses wv's slots (tag): these DMAs start once the V
                # projection has consumed wv_k (~25us in), still in time.
                # Emitted after the other phase-1 DMAs to avoid head-of-line
                # blocking in the DGE queues.
                for k in range(KO):
                    wq_t[k] = xw.tile([128, DS], f32r, tag=f"wv{k}", name=f"wq_{k}")
                    nc.sync.dma_start(wq_t[k][:, 0:128], wq[k * 128:(k + 1) * 128, 0:128])

                def proj_qk(m):
                    for w_t, x_t, b_sb, dst in (
                        (wq_t, xq_t, bq_sb, qt_all),
                        (wk_t, xk_t, bk_sb, kt_all),
                    ):
                        pss = [
                            pp1.tile([128, 512], f32, tag="p1",
                                     name=f"pj{m}{n}{dst.name[:2]}")
                            for n in range(NQ)
                        ]
                        for k in range(KO):
                            for n in range(NQ):
                                nc.tensor.matmul(
                                    pss[n][:],
                                    w_t[k][:, m * 128:(m + 1) * 128],
                                    x_t[k][:, n * 512:(n + 1) * 512],
                                    start=(k == 0), stop=(k == KO - 1),
                                )
                        for n in range(NQ):
                            # relu(x + bias) eviction -> fp32r
                            nc.vector.tensor_scalar(
                                dst[:, m, n * 512:(n + 1) * 512], pss[n][:],
                                b_sb[:, m:m + 1], 0.0, ALU.add, ALU.max,
                            )

                exp_t = [None] * NH

                def emit_scores_pair(j):
                    # heads 2j (PE rows 0-63) and 2j+1 (rows 64-127), packed
                    he, ho = 2 * j, 2 * j + 1
                    exp_t[he] = ep.tile([128, KO, L], bf16, tag="expT", name=f"expT_{he}")
                    exp_t[ho] = ep.tile([128, KO, L], bf16, tag="expT", name=f"expT_{ho}")
                    for t in range(KO):
                        pse = pp_sc.tile([128, L], f32, tag="sc", name=f"sc{he}_{t}")
                        pso = pp_sc.tile([128, L], f32, tag="sc", name=f"sc{ho}_{t}")
                        for n in range(NQ):
                            for ph, ps in ((0, pse), (DH, pso)):
                                nc.tensor.matmul(
                                    ps[:, n * 512:(n + 1) * 512],
                                    kt_all[ph:ph + DH, j, t * 128:(t + 1) * 128],
                                    qt_all[ph:ph + DH, j, n * 512:(n + 1) * 512],
                                    start=True, stop=True,
                                )
                        nc.scalar.activation(exp_t[he][:, t, :], pse[:], AF.Exp)
                        nc.scalar.activation(exp_t[ho][:, t, :], pso[:], AF.Exp)

                # head-pair 0 as early as possible
                proj_qk(0)
                emit_scores_pair(0)

                # rest of the weights, rest of the projections, scores per pair
                for k in range(KO):
                    nc.sync.dma_start(wq_t[k][:, 128:DS], wq[k * 128:(k + 1) * 128, 128:DS])
                    nc.sync.dma_start(wk_t[k][:, 128:DS], wk[k * 128:(k + 1) * 128, 128:DS])
                proj_qk(1)
                emit_scores_pair(1)
                proj_qk(2)
                proj_qk(3)

            # ================= attention tail =================
            with (
                tc.tile_pool(name="smallp", bufs=2) as smp,
                tc.tile_pool(name="outp", bufs=2) as op_,
            ):
                out_m = [None] * MQ

                def emit_ctx(h, psum_pool=None, psum_tag="p1"):
                    mh, ph = h // 2, (h % 2) * DH
                    if h % 2 == 0:
                        out_m[mh] = op_.tile([128, L], f32, tag="outm", name=f"out_{mh}")
                    pool = psum_pool if psum_pool is not None else pp1
                    pss = [
                        pool.tile([DH + 1, 512], f32, tag=psum_tag, name=f"ctx{h}_{n}")
                        for n in range(NQ)
                    ]
                    for t in range(KO):
                        for n in range(NQ):
                            nc.tensor.matmul(
                                pss[n][:],
                                v_all[:, t, h * (DH + 1):(h + 1) * (DH + 1)],
                                exp_t[h][:, t, n * 512:(n + 1) * 512],
                                start=(t == 0), stop=(t == KO - 1),
                            )
                    recip = smp.tile([1, L], f32, tag="recip", name=f"rc{h}")
                    bcast = smp.tile([DH, L], f32, tag="bcast", name=f"bc{h}")
                    for n in range(NQ):
                        nc.vector.reciprocal(
                            recip[:, n * 512:(n + 1) * 512], pss[n][DH:DH + 1, :])
                    nc.gpsimd.partition_broadcast(bcast[:], recip[:])
                    for n in range(NQ):
                        nc.vector.tensor_tensor(
                            out_m[mh][ph:ph + DH, n * 512:(n + 1) * 512],
                            pss[n][0:DH, :], bcast[:, n * 512:(n + 1) * 512],
                            ALU.mult,
                        )
                    if h % 2 == 1:
                        xqh_t = smp.tile([128, L], f32, tag="xqh", name=f"xqh_{mh}")
                        nc.sync.dma_start(
                            xqh_t[:], xqh[mh * 128:(mh + 1) * 128, :])
                        nc.vector.tensor_tensor(
                            out_m[mh][:], out_m[mh][:], xqh_t[:], ALU.add,
                        )
                        nc.sync.dma_start(
                            outT[mh * 128:(mh + 1) * 128, :], out_m[mh][:])

                emit_scores_pair(2)
                emit_ctx(0)
                emit_ctx(1)
                emit_scores_pair(3)
                emit_ctx(2)
                emit_ctx(3)
                emit_ctx(4)
                emit_ctx(5)
                emit_ctx(6)
                emit_ctx(7)

    nc.compile()
    names = {
        "xqT": xqT.name, "xkT": xkT.name, "wq": wq.name, "wk": wk.name,
        "wv": wv.name, "bq": bq.name, "bk": bk.name, "bv": bv.name,
        "ones": ones.name, "xqh": xqh.name, "outT": outT.name,
    }
    return nc, names


def _prep_in_maps(nm, queries, keys, Wq, bq, Wk, bk, Wv, bv):
    DS, DH, NH = 512, 64, 8
    in_maps = []
    for c in range(8):
        b, half = c // 2, c % 2
        sl = slice(half * DS, (half + 1) * DS)
        # interleaved augmented V weights/bias: per head 64 value cols + 1 aug
        wv_aug = np.zeros((1024, DS + NH), dtype=np.float32)
        bv_aug = np.zeros((1, DS + NH), dtype=np.float32)
        for h in range(NH):
            wv_aug[:, h * 65:h * 65 + DH] = Wv[:, half * DS + h * DH:half * DS + (h + 1) * DH]
            bv_aug[0, h * 65:h * 65 + DH] = bv[half * DS + h * DH:half * DS + (h + 1) * DH]
            bv_aug[0, h * 65 + DH] = 2.0
        in_maps.append({
            nm["xqT"]: np.ascontiguousarray(queries[b].T),
            nm["xkT"]: np.ascontiguousarray(keys[b].T),
            nm["wq"]: np.ascontiguousarray(Wq[:, sl]),
            nm["wk"]: np.ascontiguousarray(Wk[:, sl]),
            nm["wv"]: wv_aug,
            nm["bq"]: np.ascontiguousarray(bq[sl].reshape(4, 128).T),
            nm["bk"]: np.ascontiguousarray(bk[sl].reshape(4, 128).T),
            nm["bv"]: bv_aug,
            nm["ones"]: np.ones((1, 128), dtype=np.float32),
            nm["xqh"]: np.ascontiguousarray(0.5 * queries[b, :, sl].T),
        })
    return in_maps


def kernel(queries, keys, Wq, bq, Wk, bk, Wv, bv):
    import concourse.bass as bass
    import concourse.mybir as mybir
    import concourse.tile as tile
    from concourse import bacc
    from concourse.bass_utils import run_bass_kernel_spmd

    queries = np.asarray(queries, dtype=np.float32)
    keys = np.asarray(keys, dtype=np.float32)
    Wq = np.asarray(Wq, dtype=np.float32)
    Wk = np.asarray(Wk, dtype=np.float32)
    Wv = np.asarray(Wv, dtype=np.float32)
    bq = np.asarray(bq, dtype=np.float32)
    bk = np.asarray(bk, dtype=np.float32)
    bv = np.asarray(bv, dtype=np.float32)

    B, L, D = queries.shape
    DS = 512

    nc, nm = _build((bass, mybir, tile, bacc))
    in_maps = _prep_in_maps(nm, queries, keys, Wq, bq, Wk, bk, Wv, bv)
    res = run_bass_kernel_spmd(nc, in_maps, core_ids=list(range(8)))

    out = np.empty((B, L, D), dtype=np.float32)
    for c in range(8):
        b, half = c // 2, c % 2
        out[b, :, half * DS:(half + 1) * DS] = res.results[c][nm["outT"]].T
    return out
